# revision 8
# baseline (speedup 1.0000x reference)
"""Trainium2 Bass kernel for nn_CrossAttention_16441134809459.

Contract: kernel(**inputs) takes FULL unsharded inputs (numpy/jax arrays,
keys as in reference.setup_inputs()) and returns the FULL output
[8, 320, 32, 32] float32.

Sharding: data-parallel over batch — batch=8, one batch element per
NeuronCore, no collectives. Each core runs a fused cross-attention:

  q = w_q @ x_q            [512, 1024]   (1x1 conv == channel matmul)
  k = w_k @ x_kv           [512, 1024]
  vT = (w_v @ x_kv).T      [1024, 512]   (computed directly transposed:
                                          lhsT = x_kv, rhs = w_v.T)
  per head h (d=64):
    simT[j,i] = k[h].T @ q[h]   -- scores TRANSPOSED (keys on partitions)
    e = exp(simT * 1/8)          -- ACT, scale folded into the activation
    [num; den] = [vT_h | 1].T @ e   -- M=65 matmul: row 64 = softmax denom
    hidden[h*64+d, i] = num[d,i] * (1/den[i])  -- K=1 PE broadcast + DVE mult
  y = w_out @ hidden       [320, 1024]

Softmax max-subtraction is skipped: logits are ~N(0,1) (max over 8.4M
samples ~5.6), exp never overflows in fp32, and softmax is shift-invariant.
"""

import numpy as np

HEADS = 8
D = 64
HIDDEN = 512
QD = 320
KVD = 640
N = 1024
NCORES = 8

_cache = {}


def _build():
    import concourse.mybir as mybir
    import concourse.tile as tile
    from concourse import bacc
    from contextlib import ExitStack

    dt = mybir.dt.float32
    Exp = mybir.ActivationFunctionType.Exp
    mult = mybir.AluOpType.mult

    # Bacc (not raw Bass): its compile() pass splits sync waits to satisfy
    # the TRN2 per-instruction wait limits (<=1, EVSEM <=2) and moves matmul
    # waits onto LDWEIGHTS.
    nc = bacc.Bacc()
    xq_d = nc.declare_dram_parameter("x_q", [QD, N], dt, isOutput=False)
    xkv_d = nc.declare_dram_parameter("x_kv", [KVD, N], dt, isOutput=False)
    wqT_d = nc.declare_dram_parameter("w_qT", [QD, HIDDEN], dt, isOutput=False)
    wkT_d = nc.declare_dram_parameter("w_kT", [KVD, HIDDEN], dt, isOutput=False)
    wvT_d = nc.declare_dram_parameter("w_vT", [KVD, HIDDEN], dt, isOutput=False)
    woT_d = nc.declare_dram_parameter("w_oT", [HIDDEN, QD], dt, isOutput=False)
    y_d = nc.declare_dram_parameter("y", [QD, N], dt, isOutput=True)

    with tile.TileContext(nc) as tc:
        with ExitStack() as ctx:
            singles = ctx.enter_context(tc.tile_pool(name="singles", bufs=1))
            # x_q / x_kv / per-head exp tiles share one 2-slot rotation:
            # the inputs are consumed by the projections before the first
            # exp tile needs a slot.
            big = ctx.enter_context(tc.tile_pool(name="big", bufs=2))
            bcp = ctx.enter_context(tc.tile_pool(name="bcp", bufs=2))
            yst = ctx.enter_context(tc.tile_pool(name="yst", bufs=2))
            otp = ctx.enter_context(tc.tile_pool(name="otp", bufs=2))
            utlp = ctx.enter_context(tc.tile_pool(name="utl", bufs=2))
            # PSUM budget (8 banks): big 2x[128,1024]=4, o 1x[65,1024]=2,
            # m 2x[128,512]=2
            ps_big = ctx.enter_context(tc.tile_pool(name="ps_big", bufs=2, space="PSUM"))
            ps_o = ctx.enter_context(tc.tile_pool(name="ps_o", bufs=1, space="PSUM"))
            ps_m = ctx.enter_context(tc.tile_pool(name="ps_m", bufs=2, space="PSUM"))

            # persistent SBUF tensors
            wqT = singles.tile([128, 3, HIDDEN], dt)   # w_q.T, K=320 padded to 384
            wkT = singles.tile([128, 5, HIDDEN], dt)   # w_k.T
            wvT = singles.tile([128, 5, HIDDEN], dt)   # w_v.T (rhs for vT proj)
            woT = singles.tile([128, 4, QD], dt)       # w_out.T
            q_sb = singles.tile([128, 4, N], dt)       # q channels x i
            k_sb = singles.tile([128, 4, N], dt)       # k channels x j
            vt_sb = singles.tile([128, 8, HEADS * (D + 1)], dt)  # [j, (h,65)]
            hid = singles.tile([128, 4, N], dt)        # attention out, channels x i
            ones_sb = singles.tile([128, D], dt)       # row 64 used as K=1 lhsT

            nc.vector.memset(ones_sb[:], 1.0)
            nc.vector.memset(vt_sb[:], 1.0)            # ones columns survive
            nc.vector.memset(wqT[64:128, 2, :], 0.0)   # zero-pad K chunk 2

            x_q = big.tile([128, 3, N], dt, tag="big")
            x_kv = big.tile([128, 5, N], dt, tag="big")
            nc.vector.memset(x_q[64:128, 2, :], 0.0)

            # loads
            for c in range(3):
                r = 128 if c < 2 else 64
                nc.sync.dma_start(out=x_q[:r, c, :], in_=xq_d[c * 128:c * 128 + r, :])
                nc.sync.dma_start(out=wqT[:r, c, :], in_=wqT_d[c * 128:c * 128 + r, :])
            for c in range(5):
                nc.sync.dma_start(out=x_kv[:, c, :], in_=xkv_d[c * 128:(c + 1) * 128, :])
                nc.sync.dma_start(out=wkT[:, c, :], in_=wkT_d[c * 128:(c + 1) * 128, :])
                nc.sync.dma_start(out=wvT[:, c, :], in_=wvT_d[c * 128:(c + 1) * 128, :])
            for c in range(4):
                nc.sync.dma_start(out=woT[:, c, :], in_=woT_d[c * 128:(c + 1) * 128, :])

            # Q projection: q = w_q @ x_q -> [512, 1024]
            for mc in range(4):
                ps = ps_big.tile([128, N], dt, tag="big")
                for ic in range(2):
                    isl = slice(ic * 512, (ic + 1) * 512)
                    for kc in range(3):
                        nc.tensor.matmul(
                            ps[:, isl],
                            wqT[:, kc, mc * 128:(mc + 1) * 128],
                            x_q[:, kc, isl],
                            start=(kc == 0), stop=(kc == 2))
                nc.vector.tensor_copy(out=q_sb[:, mc, :], in_=ps[:, :])

            # K projection: k = w_k @ x_kv -> [512, 1024]
            for mc in range(4):
                ps = ps_big.tile([128, N], dt, tag="big")
                for ic in range(2):
                    isl = slice(ic * 512, (ic + 1) * 512)
                    for kc in range(5):
                        nc.tensor.matmul(
                            ps[:, isl],
                            wkT[:, kc, mc * 128:(mc + 1) * 128],
                            x_kv[:, kc, isl],
                            start=(kc == 0), stop=(kc == 4))
                nc.vector.tensor_copy(out=k_sb[:, mc, :], in_=ps[:, :])

            # vT projection: vT = x_kv.T @ w_v.T -> [1024 j, 512],
            # scattered into 65-wide per-head blocks (col 64 stays 1.0)
            for jc in range(8):
                ps = ps_m.tile([128, 512], dt, tag="m")
                for kc in range(5):
                    nc.tensor.matmul(
                        ps[:, :],
                        x_kv[:, kc, jc * 128:(jc + 1) * 128],
                        wvT[:, kc, :],
                        start=(kc == 0), stop=(kc == 4))
                nc.vector.tensor_copy(
                    out=vt_sb[:, jc].rearrange("p (h e) -> p h e", e=D + 1)[:, :, :D],
                    in_=ps.rearrange("p (h d) -> p h d", d=D))

            # attention, heads in pairs (even head on partitions 0-63, odd on
            # 64-127 -> K=64 matmuls land on disjoint PE row groups and
            # overlap)
            for hp in range(4):
                pair = (2 * hp, 2 * hp + 1)
                exp_t = {h: big.tile([128, 8, N], dt, tag="big", name=f"exp{h}")
                         for h in pair}
                for jc in range(8):
                    for h in pair:
                        poff = (h % 2) * 64
                        hc = h // 2
                        ps = ps_big.tile([128, N], dt, tag="big")
                        for ic in range(2):
                            isl = slice(ic * 512, (ic + 1) * 512)
                            nc.tensor.matmul(
                                ps[:, isl],
                                k_sb[poff:poff + 64, hc, jc * 128:(jc + 1) * 128],
                                q_sb[poff:poff + 64, hc, isl],
                                start=True, stop=True)
                        nc.scalar.activation(
                            out=exp_t[h][:, jc, :], in_=ps[:, :], func=Exp,
                            scale=0.125)

                for h in pair:
                    hc = h // 2
                    # [num; den] accumulated over j chunks; row 64 = denom
                    ps_ot = ps_o.tile([65, N], dt, tag="o")
                    for ic in range(2):
                        isl = slice(ic * 512, (ic + 1) * 512)
                        for jc in range(8):
                            nc.tensor.matmul(
                                ps_ot[:, isl],
                                vt_sb[:, jc, h * 65:(h + 1) * 65],
                                exp_t[h][:, jc, isl],
                                start=(jc == 0), stop=(jc == 7))
                    util = utlp.tile([128, N], dt, tag="u")
                    otemp = (otp.tile([64, N], dt, tag="ot", name=f"ot{h}")
                             if h % 2 else None)
                    for ic in range(2):
                        isl = slice(ic * 512, (ic + 1) * 512)
                        nc.vector.reciprocal(out=util[64:65, isl], in_=ps_ot[64:65, isl])
                        # broadcast recip across partitions: K=1 matmul from
                        # partition 64 (row group 2), ones x recip
                        ps_b = ps_m.tile([64, 512], dt, tag="m")
                        nc.tensor.matmul(
                            ps_b[:, :], ones_sb[64:65, :], util[64:65, isl],
                            start=True, stop=True)
                        bc = bcp.tile([64, 512], dt, tag="bc")
                        nc.vector.tensor_copy(out=bc[:, :], in_=ps_b[:, :])
                        if h % 2 == 0:
                            nc.vector.tensor_tensor(
                                hid[0:64, hc, isl], ps_ot[0:64, isl], bc[:, :], mult)
                        else:
                            nc.vector.tensor_tensor(
                                otemp[:, isl], ps_ot[0:64, isl], bc[:, :], mult)
                    if h % 2:
                        # DVE lanes can't shift partitions; DMA moves the odd
                        # head's rows into partitions 64-127 of the hidden tile
                        nc.sync.dma_start(out=hid[64:128, hc, :], in_=otemp[:, :])

            # output projection: y = w_out @ hidden -> [320, 1024]
            for mc in range(3):
                msz = 128 if mc < 2 else 64
                for ic in range(2):
                    isl = slice(ic * 512, (ic + 1) * 512)
                    ps = ps_m.tile([128, 512], dt, tag="m")
                    for kc in range(4):
                        nc.tensor.matmul(
                            ps[:msz, :],
                            woT[:, kc, mc * 128:mc * 128 + msz],
                            hid[:, kc, isl],
                            start=(kc == 0), stop=(kc == 3))
                    yt = yst.tile([128, 512], dt, tag="y")
                    nc.vector.tensor_copy(out=yt[:msz, :], in_=ps[:msz, :])
                    nc.sync.dma_start(out=y_d[mc * 128:mc * 128 + msz, isl],
                                      in_=yt[:msz, :])

    nc.compile()
    return nc


def _get_nc():
    if "nc" not in _cache:
        _cache["nc"] = _build()
    return _cache["nc"]


def _run(inputs, trace=False):
    from concourse.bass_utils import run_bass_kernel_spmd

    nc = _get_nc()
    x_q = np.asarray(inputs["x_q"], dtype=np.float32).reshape(8, QD, N)
    x_kv = np.asarray(inputs["x_kv"], dtype=np.float32).reshape(8, KVD, N)
    w_q = np.asarray(inputs["w_q"], dtype=np.float32)
    w_kv = np.asarray(inputs["w_kv"], dtype=np.float32)
    w_out = np.asarray(inputs["w_out"], dtype=np.float32)
    w_qT = np.ascontiguousarray(w_q.T)
    w_kT = np.ascontiguousarray(w_kv[:HIDDEN].T)
    w_vT = np.ascontiguousarray(w_kv[HIDDEN:].T)
    w_oT = np.ascontiguousarray(w_out.T)
    in_maps = [
        {"x_q": np.ascontiguousarray(x_q[b]),
         "x_kv": np.ascontiguousarray(x_kv[b]),
         "w_qT": w_qT, "w_kT": w_kT, "w_vT": w_vT, "w_oT": w_oT}
        for b in range(NCORES)
    ]
    res = run_bass_kernel_spmd(nc, in_maps, core_ids=list(range(NCORES)),
                               trace=trace)
    y = np.stack([res.results[b]["y"] for b in range(NCORES)])
    return y.reshape(8, QD, 32, 32), res


def kernel(**inputs):
    y, _ = _run(inputs)
    return y
